# revision 2
# baseline (speedup 1.0000x reference)
"""Trainium2 Bass kernel for nn_DistanceRestraint (histogram_binning), v7.

Strategy (8 NeuronCores, SPMD):
  - Host dedups the 262144 pairs into unique (i,j) cells with multiplicities;
    cells are routed to 8 cores by flat id (i*1024+j), 131072 table cells per
    core, 4 windows of 32768 cells so gather indices fit int16.
  - Host builds a data-independent "mega" table [L*L, 128] f32 (512 B rows):
    12 f32 CB[:,i] + 12 f32 CB[:,j] + 32 fp16 spline coeffs (segments 0..7,
    s-major, c-fast) packed as 16 f32 "carriers".  fp16 coeffs that would be
    subnormal are clamped to the minimal normal so a carrier f32 is never
    subnormal (flush-to-zero on the mult path would corrupt its partner).
  - Device: per iteration (2 windows = 128 slot columns), dma_gather fetches
    the occupied cells' 512B rows (8x 1024-idx calls per window); DVE computes
    distances (diff^2 + Act square/sqrt), bins EXACTLY by comparing d^2
    against k^2 (7-op is_gt chain), selects segment coefficients with
    is_equal STT on f32 carriers (half the writes of an fp16 select), sums
    segments via a pairwise-add tree on the fp16 bitcast view (2x DVE mode),
    Horner-evaluates in fp16, and accumulates multiplicity-weighted sums in
    f32 via STT accum_out.
  - Each core returns 128 partial sums; host reduces in float64.

Assumptions validated host-side (raise if violated):
  - unit-spaced cutoffs 0..36;
  - all distances < 8 (S=8 segments kept; P(d>=8) ~ 5e-7 per sample for
    N(0,1) CB data; known RNG streams peak at d~7.4);
  - <= 8192 occupied cells per (core, window) (mean ~7250, sigma ~76).
With max d < 8 < 36 the reference's d<=cutoffs[-1] mask is a no-op here; it
is folded into the multiplicity weights (padding slots carry weight 0).
"""
import numpy as np

import concourse.bacc as bacc
import concourse.mybir as mybir
import concourse.tile as tile
from concourse import bass_utils

L = 1024
B = 4
S = 8                  # spline segments kept in the mega table
ROWF = 128             # floats per mega row (512 B)
NC = 8                 # NeuronCores
CELLS = (L * L) // NC  # table cells per core
WINDOW = 32768         # cells per int16 index window
NWIN = CELLS // WINDOW            # 4
NQW = 8192                        # padded cell slots per window
COLS_W = NQW // 128               # 64 per-partition columns per window
CALL = 1024                       # gather indices per dma_gather call
NCALL_W = NQW // CALL             # 8
IDXC_W = NQW // 16                # 512 idx columns per window
IDXCOLS = NWIN * IDXC_W           # 2048
WPI = 1                           # windows per compute iteration
NITER = NWIN // WPI               # 2
M = WPI * COLS_W                  # 128 slot columns per iteration

_NC_CACHE = {}


def _build_module():
    if "nc" in _NC_CACHE:
        return _NC_CACHE["nc"]
    nc = bacc.Bacc("TRN2", target_bir_lowering=False, debug=False, num_devices=NC,
                   num_swdge_queues=4)

    mega = nc.dram_tensor("mega", [CELLS, ROWF], mybir.dt.float32, kind="ExternalInput")
    idx16 = nc.dram_tensor("idx16", [128, IDXCOLS], mybir.dt.int16, kind="ExternalInput")
    multw = nc.dram_tensor("multw", [128, NWIN * COLS_W], mybir.dt.float32,
                           kind="ExternalInput")
    acc_out = nc.dram_tensor("acc_out", [128, 2], mybir.dt.float32, kind="ExternalOutput")

    f32 = mybir.dt.float32
    f16 = mybir.dt.float16
    Alu = mybir.AluOpType
    Act = mybir.ActivationFunctionType

    with tile.TileContext(nc) as tc:
        with tc.tile_pool(name="const", bufs=1) as cpool, \
             tc.tile_pool(name="g", bufs=3) as gpool, \
             tc.tile_pool(name="w", bufs=1) as wpool:
            t_idx = cpool.tile([128, IDXCOLS], mybir.dt.int16)
            for w in range(NWIN):
                nc.sync.dma_start(
                    out=t_idx[:, w * IDXC_W:(w + 1) * IDXC_W],
                    in_=idx16.ap()[:, w * IDXC_W:(w + 1) * IDXC_W])
            t_mult = cpool.tile([128, NWIN * COLS_W], f32)
            nc.sync.dma_start(out=t_mult[:], in_=multw.ap())
            t_acc = cpool.tile([128, 2], f32)
            nc.vector.memset(t_acc[:], 0.0)
            t_zero = cpool.tile([128, COLS_W, B], f32)
            nc.vector.memset(t_zero[:], 0.0)

            def emit_compute(Gc, mcol0, Mc, tg, eng, acc):
                A = Gc[:, :, 0:12]
                Bv = Gc[:, :, 12:24]

                diff = wpool.tile([128, Mc, 12], f32, tag="diff" + tg)
                eng.tensor_tensor(out=diff[:], in0=A, in1=Bv, op=Alu.subtract)
                dsq = wpool.tile([128, Mc, 12], f32, tag="dsq" + tg)
                nc.scalar.activation(dsq[:], diff[:], Act.Square)
                ssum = wpool.tile([128, Mc, B], f32, tag="ssum" + tg)
                dsq4 = dsq[:].rearrange("p m (b k) -> p m b k", k=3)
                if eng is nc.vector:
                    eng.tensor_reduce(
                        out=ssum[:], in_=dsq4,
                        axis=mybir.AxisListType.X, op=Alu.add)
                else:
                    eng.tensor_tensor(out=ssum[:], in0=dsq4[:, :, :, 0],
                                      in1=dsq4[:, :, :, 1], op=Alu.add)
                    eng.tensor_tensor(out=ssum[:], in0=ssum[:],
                                      in1=dsq4[:, :, :, 2], op=Alu.add)

                d0 = wpool.tile([128, Mc, B], f32, tag="d0" + tg)
                nc.scalar.activation(d0[:], ssum[:], Act.Sqrt)

                # idx = #{k in 1..7 : d^2 > k^2} -- exact bin edges
                idxf = wpool.tile([128, Mc, B], f32, tag="idxf" + tg)
                eng.scalar_tensor_tensor(
                    out=idxf[:], in0=ssum[:], scalar=1.0,
                    in1=t_zero[:, :Mc, :], op0=Alu.is_gt, op1=Alu.add)
                for k in range(2, S):
                    eng.scalar_tensor_tensor(
                        out=idxf[:], in0=ssum[:], scalar=float(k * k),
                        in1=idxf[:], op0=Alu.is_gt, op1=Alu.add)

                xr = wpool.tile([128, Mc, B], f32, tag="xr" + tg)
                eng.tensor_tensor(out=xr[:], in0=d0[:], in1=idxf[:],
                                        op=Alu.subtract)
                xr16 = wpool.tile([128, Mc, B], f16, tag="xr16" + tg)
                nc.scalar.activation(xr16[:], xr[:], Act.Copy)

                # select the chosen segment's 4 coeffs: is_equal against the
                # repeated-carrier row block (8 f32 per segment), all ops 3D
                idxE2 = wpool.tile([128, Mc, 2, B], f32, tag="idxE2" + tg)
                nc.scalar.activation(idxE2[:, :, 0, :], idxf[:], Act.Copy)
                nc.scalar.activation(idxE2[:, :, 1, :], idxf[:], Act.Copy)
                T5 = wpool.tile([128, Mc, 2, B, S], f32, tag="T5" + tg)
                TV = T5[:].rearrange("p m c b s -> p m (c b) s")
                iE3 = idxE2[:].rearrange("p m c b -> p m (c b)")
                for s in range(S):
                    eng.scalar_tensor_tensor(
                        out=TV[:, :, :, s], in0=iE3, scalar=float(s),
                        in1=Gc[:, :, 28 + 8 * s:28 + 8 * s + 8],
                        op0=Alu.is_equal, op1=Alu.mult)

                # segment-sum tree on the fp16 view ((m,c,b) merged, 2x mode)
                V = T5[:].bitcast(f16).rearrange("p m c b s -> p (m c b) s")
                t4 = wpool.tile([128, Mc * 2 * B, S], f16, tag="t4" + tg)
                eng.tensor_tensor(out=t4[:], in0=V[:, :, 0:S],
                                        in1=V[:, :, S:2 * S], op=Alu.add)
                t2 = wpool.tile([128, Mc * 2 * B, S // 2], f16, tag="t2" + tg)
                eng.tensor_tensor(out=t2[:], in0=t4[:, :, 0:S // 2],
                                        in1=t4[:, :, S // 2:S], op=Alu.add)
                csel = wpool.tile([128, Mc, 2, B, 2], f16, tag="csel" + tg)
                csel3 = csel[:].rearrange("p m c b s -> p (m c b) s")
                eng.tensor_tensor(out=csel3, in0=t2[:, :, 0:2],
                                        in1=t2[:, :, 2:4], op=Alu.add)

                # Horner in fp16; coeff c at csel[:, :, c//2, :, c%2]
                h = wpool.tile([128, Mc, B], f16, tag="h" + tg)
                eng.tensor_tensor(out=h[:], in0=csel[:, :, 0, :, 0],
                                        in1=xr16[:], op=Alu.mult)
                eng.tensor_tensor(out=h[:], in0=h[:], in1=csel[:, :, 0, :, 1],
                                        op=Alu.add)
                eng.tensor_tensor(out=h[:], in0=h[:], in1=xr16[:], op=Alu.mult)
                eng.tensor_tensor(out=h[:], in0=h[:], in1=csel[:, :, 1, :, 0],
                                        op=Alu.add)
                eng.tensor_tensor(out=h[:], in0=h[:], in1=xr16[:], op=Alu.mult)
                eng.tensor_tensor(out=h[:], in0=h[:], in1=csel[:, :, 1, :, 1],
                                        op=Alu.add)

                h32 = wpool.tile([128, Mc, B], f32, tag="h32" + tg)
                nc.scalar.activation(h32[:], h[:], Act.Copy)

                mult_bc = t_mult[:, mcol0:mcol0 + Mc].rearrange(
                    "p (m b) -> p m b", b=1).to_broadcast([128, Mc, B])
                wsum = wpool.tile([128, Mc, B], f32, tag="wsum" + tg)
                if eng is nc.vector:
                    r1 = wpool.tile([128, 1], f32, tag="r1" + tg)
                    eng.scalar_tensor_tensor(
                        out=wsum[:], in0=h32[:], scalar=1.0, in1=mult_bc,
                        op0=Alu.mult, op1=Alu.mult, accum_out=r1[:])
                    eng.tensor_tensor(out=acc, in0=acc, in1=r1[:], op=Alu.add)
                else:
                    eng.scalar_tensor_tensor(
                        out=wsum[:], in0=h32[:], scalar=1.0, in1=mult_bc,
                        op0=Alu.mult, op1=Alu.mult)
                    r1 = wpool.tile([128, 1], f32, tag="r1" + tg)
                    nc.vector.tensor_reduce(out=r1[:], in_=wsum[:],
                                            axis=mybir.AxisListType.XY,
                                            op=Alu.add)
                    nc.vector.tensor_tensor(out=acc, in0=acc, in1=r1[:],
                                            op=Alu.add)

            for w in range(NWIN):
                G = gpool.tile([128, COLS_W, ROWF], f32, tag="G")
                for q in range(NCALL_W):
                    nc.gpsimd.dma_gather(
                        out_ap=G[:, q * (CALL // 128):(q + 1) * (CALL // 128), :],
                        in_ap=mega.ap()[w * WINDOW:(w + 1) * WINDOW],
                        idxs_ap=t_idx[:, w * IDXC_W + q * (CALL // 16):
                                      w * IDXC_W + (q + 1) * (CALL // 16)],
                        num_idxs=CALL,
                        num_idxs_reg=CALL,
                        elem_size=ROWF,
                        queue_num=(w * NCALL_W + q) % 4,
                    )
                if w < NWIN - 1:
                    emit_compute(G[:], w * COLS_W, COLS_W, "a", nc.vector,
                                 t_acc[:, 0:1])
                else:
                    half = COLS_W // 2
                    emit_compute(G[:, 0:half, :], w * COLS_W, half, "b",
                                 nc.vector, t_acc[:, 0:1])
                    emit_compute(G[:, half:COLS_W, :], w * COLS_W + half, half,
                                 "c", nc.vector, t_acc[:, 1:2])

            nc.sync.dma_start(out=acc_out.ap(), in_=t_acc[:])
    nc.compile()
    _NC_CACHE["nc"] = nc
    return nc


def _prepare_inputs(CB, coeff, pair_i, pair_j):
    CB = np.asarray(CB, dtype=np.float32)
    coeff = np.asarray(coeff, dtype=np.float32)
    pi = np.asarray(pair_i).astype(np.int64)
    pj = np.asarray(pair_j).astype(np.int64)

    d = np.linalg.norm(CB[:, pi] - CB[:, pj], axis=-1)
    if d.max() >= float(S):
        raise RuntimeError(f"max distance {d.max():.3f} >= {S}: table too small")

    T1 = np.ascontiguousarray(CB.transpose(1, 0, 2).reshape(L, 3 * B))
    mega = np.zeros((L * L, ROWF), dtype=np.float32)
    mega[:, 0:12] = np.repeat(T1, L, axis=0)
    mega[:, 12:24] = np.tile(T1, (L, 1))
    c16 = coeff[:, :, :S, :].reshape(L * L, 4 * S).astype(np.float16)
    # clamp subnormal/zero fp16 away from zero so no f32 carrier is subnormal
    min_norm = np.float16(6.104e-5)
    tiny = np.abs(c16) < min_norm
    c16 = np.where(tiny, np.where(c16 < 0, -min_norm, min_norm), c16)
    # each f32 carrier (2 packed fp16 coeffs) repeated 4x: [s][cc][rep4]
    mega[:, 28:92] = np.repeat(c16.view(np.float32), 4, axis=1)

    flat = pi * L + pj
    cells, mult = np.unique(flat, return_counts=True)
    core = cells // CELLS
    win = (cells % CELLS) // WINDOW
    local = (cells % WINDOW).astype(np.int64)
    bucket = core * NWIN + win
    counts = np.bincount(bucket, minlength=NC * NWIN)
    if counts.max() > NQW:
        raise RuntimeError(f"window overflow: max {counts.max()} > {NQW}")
    starts = np.zeros(NC * NWIN, dtype=np.int64)
    starts[1:] = np.cumsum(counts)[:-1]
    slot = np.arange(len(cells)) - starts[bucket]  # slot within (core, win)

    idx_arr = np.zeros((NC, 16, IDXCOLS), dtype=np.int16)
    mult_arr = np.zeros((NC, 128, NWIN * COLS_W), dtype=np.float32)

    idx_arr[core, slot % 16, win * IDXC_W + slot // 16] = local.astype(np.int16)
    mult_arr[core, slot % 128, win * COLS_W + slot // 128] = mult.astype(np.float32)

    in_maps = []
    for c in range(NC):
        in_maps.append({
            "mega": mega[c * CELLS:(c + 1) * CELLS],
            "idx16": np.tile(idx_arr[c], (8, 1)),
            "multw": mult_arr[c],
        })
    return in_maps


def kernel(CB, coeff, cutoffs, pair_i, pair_j):
    cutoffs = np.asarray(cutoffs, dtype=np.float32)
    if not np.array_equal(cutoffs, np.arange(len(cutoffs), dtype=np.float32)):
        raise NotImplementedError("kernel assumes unit-spaced cutoffs starting at 0")
    nc = _build_module()
    in_maps = _prepare_inputs(CB, coeff, pair_i, pair_j)
    res = bass_utils.run_bass_kernel_spmd(nc, in_maps, core_ids=list(range(NC)))
    total = np.float64(0.0)
    for r in res.results:
        total += r["acc_out"].astype(np.float64).sum()
    return np.float32(total)
